# revision 1
# baseline (speedup 1.0000x reference)
"""Trainium2 Bass kernel for PhylogeneticGNN (3-layer GCN trunk + folded head).

Strategy (8 NeuronCores, SPMD):
  - Nodes sharded 6250/core (padded to 6272 = 49*128); edges partitioned by
    destination core, sorted by destination tile, padded to uniform chunks
    of 128 edges.
  - Per conv layer: dense q = h @ W on the local shard (PE), bf16 AllGather
    of q (25.7MB), indirect-DMA gather of source rows (512B each), then
    aggregation as S-matrix matmuls accumulated in fp32 PSUM, where
    S[e, dest_slot(e)] = norm(e) is host-precomputed per 128-edge chunk.
  - Orientation keeps h feature-major on chip (zero transposes): dense uses
    lhsT = hT tile, agg uses lhsT = gathered rows, rhs = S.
  - The final conv layer + output projection + head only affect the single
    target row, so they are folded algebraically onto the host: the device
    returns h2T (fp32) and the host computes
    h3[t] = (sum_e norm_e h2[src_e]) @ Wg[2] + bg[2], then the MLP head.
"""

import os
import sys

import numpy as np

if "/opt/trn_rl_repo" not in sys.path:
    sys.path.insert(0, "/opt/trn_rl_repo")

import ml_dtypes

from concourse import bacc, bass, mybir, tile
from concourse.bass_utils import run_bass_kernel_spmd

BF16 = ml_dtypes.bfloat16

N = 50000
E = 300000
F_IN = 100
H = 256
NC = 8
P = 128
NSH = N // NC            # 6250 real nodes per core
NT = (NSH + P - 1) // P  # 49 dest tiles per core
NP = NT * P              # 6272 padded nodes per core
NPROJ = 14               # input-proj node chunks
WPROJ = NP // NPROJ      # 448

_PROGRAM_CACHE = {}


def _build_program(C: int):
    """Build the SPMD Bass program for chunk-depth C (chunks per dest tile)."""
    f32 = mybir.dt.float32
    bf16 = mybir.dt.bfloat16
    i32 = mybir.dt.int32
    NCHUNK = NT * C

    nc = bacc.Bacc("TRN2", target_bir_lowering=False, debug=False, num_devices=NC)

    xT_d = nc.dram_tensor("xT", [F_IN, NP], bf16, kind="ExternalInput")
    wi_d = nc.dram_tensor("WI", [F_IN, H], bf16, kind="ExternalInput")
    bi_d = nc.dram_tensor("BI", [P, 2], f32, kind="ExternalInput")
    wg_d = nc.dram_tensor("WG", [P, 4 * H], bf16, kind="ExternalInput")
    bg_d = nc.dram_tensor("BG", [P, 4], f32, kind="ExternalInput")
    s_d = nc.dram_tensor("S", [P, NCHUNK * P], bf16, kind="ExternalInput")
    idx_d = nc.dram_tensor("IDX", [P, NCHUNK], i32, kind="ExternalInput")
    h2t_d = nc.dram_tensor("H2T", [H, NP], f32, kind="ExternalOutput")

    rg = [list(range(NC))]

    with tile.TileContext(nc) as tc:
        with (
            tc.tile_pool(name="const", bufs=1) as const,
            tc.tile_pool(name="dram", bufs=1, space="DRAM") as dram,
            tc.tile_pool(name="qpool", bufs=4) as qpool,
            tc.tile_pool(name="gpool", bufs=12) as gpool,
            tc.tile_pool(name="psum", bufs=2, space="PSUM") as psum,
        ):
            xT_sb = const.tile([F_IN, NP], bf16)
            wi_sb = const.tile([F_IN, H], bf16)
            bi_sb = const.tile([P, 2], f32)
            wg_sb = const.tile([P, 4 * H], bf16)
            bg_sb = const.tile([P, 4], f32)
            s_sb = const.tile([P, NCHUNK * P], bf16)
            idx_sb = const.tile([P, NCHUNK], i32)
            h0T = const.tile([P, NP], bf16)
            h1T = const.tile([P, NP], bf16)
            hT = (h0T, h1T)

            cc0 = dram.tile([NP, H], bf16)
            cc1 = dram.tile([NP, H], bf16)
            qf0 = dram.tile([NC * NP, H], bf16, addr_space="Shared")
            qf1 = dram.tile([NC * NP, H], bf16, addr_space="Shared")

            nc.sync.dma_start(out=wi_sb[:], in_=wi_d[:])
            nc.sync.dma_start(out=bi_sb[:], in_=bi_d[:])
            nc.sync.dma_start(out=wg_sb[:], in_=wg_d[:])
            nc.sync.dma_start(out=bg_sb[:], in_=bg_d[:])
            nc.sync.dma_start(out=xT_sb[:], in_=xT_d[:])
            nc.sync.dma_start(out=idx_sb[:], in_=idx_d[:])
            nc.sync.dma_start(out=s_sb[:], in_=s_d[:])

            # ---- input projection: h0T = relu(Wi.T @ xT + bi) -------------
            for j in range(NPROJ):
                cs = slice(j * WPROJ, (j + 1) * WPROJ)
                for m in range(2):
                    ps_proj = psum.tile([P, WPROJ], f32)
                    nc.tensor.matmul(
                        out=ps_proj[:],
                        lhsT=wi_sb[:, m * P : (m + 1) * P],
                        rhs=xT_sb[:, cs],
                        start=True,
                        stop=True,
                    )
                    nc.scalar.activation(
                        out=hT[m][:, cs],
                        in_=ps_proj[:],
                        func=mybir.ActivationFunctionType.Relu,
                        bias=bi_sb[:, m : m + 1],
                        scale=1.0,
                    )

            # ---- two full GCN conv layers ---------------------------------
            for layer in range(2):
                cc_in = (cc0, cc1)[layer]
                qf = (qf0, qf1)[layer]

                # dense: q = h @ Wg[layer]  (node-major out, bf16, to DRAM)
                for nt in range(NT):
                    ps_q = psum.tile([P, H], f32)
                    for fc in range(2):
                        nc.tensor.matmul(
                            out=ps_q[:],
                            lhsT=hT[fc][:, nt * P : (nt + 1) * P],
                            rhs=wg_sb[:, (2 * layer + fc) * H : (2 * layer + fc + 1) * H],
                            start=(fc == 0),
                            stop=(fc == 1),
                        )
                    q_sb = qpool.tile([P, H], bf16)
                    nc.vector.tensor_copy(out=q_sb[:], in_=ps_q[:])
                    nc.sync.dma_start(
                        out=cc_in[nt * P : (nt + 1) * P, :], in_=q_sb[:]
                    )

                nc.gpsimd.collective_compute(
                    "AllGather",
                    mybir.AluOpType.bypass,
                    replica_groups=rg,
                    ins=[cc_in.opt()],
                    outs=[qf.opt()],
                )

                # gather + aggregate: h_next.T[:, tile] = sum_k Qg_k.T @ S_k
                for t in range(NT):
                    ps_a = psum.tile([P, P], f32)
                    ps_b = psum.tile([P, P], f32)
                    for k in range(C):
                        jj = t * C + k
                        g = gpool.tile([P, H], bf16)
                        nc.gpsimd.indirect_dma_start(
                            out=g[:],
                            out_offset=None,
                            in_=qf[:],
                            in_offset=bass.IndirectOffsetOnAxis(
                                ap=idx_sb[:, jj : jj + 1], axis=0
                            ),
                        )
                        nc.tensor.matmul(
                            out=ps_a[:],
                            lhsT=g[:, 0:P],
                            rhs=s_sb[:, jj * P : (jj + 1) * P],
                            start=(k == 0),
                            stop=(k == C - 1),
                        )
                        nc.tensor.matmul(
                            out=ps_b[:],
                            lhsT=g[:, P : 2 * P],
                            rhs=s_sb[:, jj * P : (jj + 1) * P],
                            start=(k == 0),
                            stop=(k == C - 1),
                        )
                    ts = slice(t * P, (t + 1) * P)
                    for fc, pp in ((0, ps_a), (1, ps_b)):
                        bias_ap = bg_sb[:, 2 * layer + fc : 2 * layer + fc + 1]
                        if layer == 0:
                            nc.scalar.activation(
                                out=hT[fc][:, ts],
                                in_=pp[:],
                                func=mybir.ActivationFunctionType.Relu,
                                bias=bias_ap,
                                scale=1.0,
                            )
                        else:
                            h2_sb = qpool.tile([P, P], f32)
                            nc.scalar.activation(
                                out=h2_sb[:],
                                in_=pp[:],
                                func=mybir.ActivationFunctionType.Relu,
                                bias=bias_ap,
                                scale=1.0,
                            )
                            nc.sync.dma_start(
                                out=h2t_d[fc * P : (fc + 1) * P, ts], in_=h2_sb[:]
                            )

    nc.compile()
    return nc


def _preprocess(x, edge_index, edge_attr):
    src = np.asarray(edge_index[0], dtype=np.int64)
    dst = np.asarray(edge_index[1], dtype=np.int64)
    ew = np.asarray(edge_attr, dtype=np.float64)

    loop = np.arange(N, dtype=np.int64)
    src = np.concatenate([src, loop])
    dst = np.concatenate([dst, loop])
    ew = np.concatenate([ew, np.ones(N)])

    deg = np.zeros(N)
    np.add.at(deg, dst, ew)
    dinv = np.where(deg > 0, 1.0 / np.sqrt(np.maximum(deg, 1e-30)), 0.0)
    norm = dinv[src] * ew * dinv[dst]

    # group edges by (dest core, dest tile)
    dslot = dst % NSH
    key = (dst // NSH) * NT + dslot // P
    dcol = dslot % P
    gp_src = (src // NSH) * NP + (src % NSH)  # padded global row in Q_full

    order = np.argsort(key, kind="stable")
    key_s = key[order]
    counts = np.bincount(key_s, minlength=NC * NT)
    starts = np.zeros(NC * NT + 1, dtype=np.int64)
    np.cumsum(counts, out=starts[1:])
    pos = np.arange(len(key_s)) - starts[key_s]

    C = max(1, int(np.ceil(counts.max() / P)))
    cap = C * P
    src_pad = np.zeros((NC * NT, cap), np.int32)
    norm_pad = np.zeros((NC * NT, cap), np.float32)
    dcol_pad = np.zeros((NC * NT, cap), np.int32)
    src_pad[key_s, pos] = gp_src[order]
    norm_pad[key_s, pos] = norm[order]
    dcol_pad[key_s, pos] = dcol[order]

    return src_pad, norm_pad, dcol_pad, C, (src, dst, norm)


def _make_in_maps(x, src_pad, norm_pad, dcol_pad, C, Wi, bi, Wg, bg):
    x = np.asarray(x, dtype=np.float32)
    Wi = np.asarray(Wi, dtype=np.float32)
    bi = np.asarray(bi, dtype=np.float32)
    Wg = np.asarray(Wg, dtype=np.float32)
    bg = np.asarray(bg, dtype=np.float32)

    wi_dev = Wi.astype(BF16)
    bi_dev = np.ascontiguousarray(bi.reshape(2, P).T)
    wg_dev = np.concatenate(
        [Wg[i, fc * P : (fc + 1) * P, :] for i in range(2) for fc in range(2)],
        axis=1,
    ).astype(BF16)
    bg_dev = np.ascontiguousarray(
        np.stack(
            [bg[i, fc * P : (fc + 1) * P] for i in range(2) for fc in range(2)],
            axis=1,
        )
    )

    in_maps = []
    ar_chunk = np.arange(NT * C)[:, None]
    ar_p = np.arange(P)[None, :]
    for c in range(NC):
        rows = slice(c * NT, (c + 1) * NT)
        sp = src_pad[rows].reshape(NT * C, P)
        npd = norm_pad[rows].reshape(NT * C, P)
        dc = dcol_pad[rows].reshape(NT * C, P)
        S = np.zeros((NT * C, P, P), np.float32)
        S[ar_chunk, ar_p, dc] = npd
        s_dev = np.ascontiguousarray(S.transpose(1, 0, 2).reshape(P, NT * C * P)).astype(BF16)
        idx_dev = np.ascontiguousarray(sp.T)

        xT = np.zeros((F_IN, NP), np.float32)
        xT[:, :NSH] = x[c * NSH : (c + 1) * NSH].T
        in_maps.append(
            {
                "xT": xT.astype(BF16),
                "WI": wi_dev,
                "BI": bi_dev,
                "WG": wg_dev,
                "BG": bg_dev,
                "S": s_dev,
                "IDX": idx_dev,
            }
        )
    return in_maps


def _host_head(results, edges, t_idx, Wg, bg, Wo1, bo1, Wo2, bo2,
               Wh1, bh1, Wh2, bh2, Wh3, bh3):
    src, dst, norm = edges
    mask = dst == t_idx
    e_src = src[mask]
    e_norm = norm[mask]

    hsum = np.zeros(H, dtype=np.float64)
    for s, w in zip(e_src, e_norm):
        c = int(s // NSH)
        slot = int(s % NSH)
        hsum += w * results[c]["H2T"][:, slot].astype(np.float64)

    f8 = lambda a: np.asarray(a, dtype=np.float64)
    t = hsum @ f8(Wg)[2] + f8(bg)[2]
    t = np.maximum(t @ f8(Wo1) + f8(bo1), 0.0)
    t = t @ f8(Wo2) + f8(bo2)
    t = np.maximum(t @ f8(Wh1) + f8(bh1), 0.0)
    t = np.maximum(t @ f8(Wh2) + f8(bh2), 0.0)
    return (t @ f8(Wh3) + f8(bh3)).astype(np.float32)


def _run(inputs, trace=False, trace_kwargs=None):
    x = np.asarray(inputs["x"])
    t_idx = int(np.asarray(inputs["target_species_idx"]))

    src_pad, norm_pad, dcol_pad, C, edges = _preprocess(
        x, np.asarray(inputs["edge_index"]), np.asarray(inputs["edge_attr"])
    )

    if C not in _PROGRAM_CACHE:
        _PROGRAM_CACHE[C] = _build_program(C)
    nc = _PROGRAM_CACHE[C]

    in_maps = _make_in_maps(
        x, src_pad, norm_pad, dcol_pad, C,
        inputs["Wi"], inputs["bi"], inputs["Wg"], inputs["bg"],
    )

    res = run_bass_kernel_spmd(
        nc, in_maps, list(range(NC)), trace=trace, **(trace_kwargs or {})
    )

    out = _host_head(
        res.results, edges, t_idx,
        inputs["Wg"], inputs["bg"], inputs["Wo1"], inputs["bo1"],
        inputs["Wo2"], inputs["bo2"], inputs["Wh1"], inputs["bh1"],
        inputs["Wh2"], inputs["bh2"], inputs["Wh3"], inputs["bh3"],
    )
    return out, res


def kernel(**inputs):
    out, _ = _run(inputs)
    return out


# revision 13
# speedup vs baseline: 1.1247x; 1.1247x over previous
"""Trainium2 Bass kernel for PhylogeneticGNN (3-layer GCN trunk + folded head).

Strategy (8 NeuronCores, SPMD):
  - Nodes sharded 6250/core (padded to 6272 = 49*128); edges partitioned by
    destination core, sorted by destination tile, padded to uniform chunks
    of 128 edges.
  - Per conv layer: dense q = h @ W on the local shard (PE), bf16 AllGather
    of q (25.7MB), indirect-DMA gather of source rows (512B each), then
    aggregation as S-matrix matmuls accumulated in fp32 PSUM, where
    S[e, dest_slot(e)] = norm(e) is host-precomputed per 128-edge chunk.
  - Orientation keeps h feature-major on chip (zero transposes): dense uses
    lhsT = hT tile, agg uses lhsT = gathered rows, rhs = S.
  - The final conv layer + output projection + head only affect the single
    target row, so they are folded algebraically onto the host: the device
    returns h2T (fp32) and the host computes
    h3[t] = (sum_e norm_e h2[src_e]) @ Wg[2] + bg[2], then the MLP head.
"""

import os
import sys

import numpy as np

if "/opt/trn_rl_repo" not in sys.path:
    sys.path.insert(0, "/opt/trn_rl_repo")

import ml_dtypes

from concourse import bacc, bass, mybir, tile
from concourse.bass_utils import run_bass_kernel_spmd

BF16 = ml_dtypes.bfloat16

N = 50000
E = 300000
F_IN = 100
H = 256
NC = 8
P = 128
NSH = N // NC            # 6250 real nodes per core
NT = (NSH + P - 1) // P  # 49 dest tiles per core
NP = NT * P              # 6272 padded nodes per core
NPROJ = 14               # input-proj node chunks
WPROJ = NP // NPROJ      # 448

_PROGRAM_CACHE = {}


def _build_program(C: int, variant: str = "full", reps: int = 1):
    """Build the SPMD Bass program for chunk-depth C (chunks per dest tile).

    variant: "full" | "noindirect" (contiguous DMA instead of gather) |
             "nocoll" (skip AllGather, gather from local) |
             "noagg" (skip gather+agg matmuls) | "dense" (skip agg + AG)
    reps: repeat the conv block (timing instrument; reps>1 is wrong math)
    """
    f32 = mybir.dt.float32
    bf16 = mybir.dt.bfloat16
    i32 = mybir.dt.int32
    NCHUNK = NT * C

    nc = bacc.Bacc("TRN2", target_bir_lowering=False, debug=False, num_devices=NC)

    xT_d = nc.dram_tensor("xT", [F_IN, NP], bf16, kind="ExternalInput")
    wi_d = nc.dram_tensor("WI", [F_IN, H], bf16, kind="ExternalInput")
    bi_d = nc.dram_tensor("BI", [P, 2], f32, kind="ExternalInput")
    wg_d = nc.dram_tensor("WG", [P, 4 * H], bf16, kind="ExternalInput")
    bg_d = nc.dram_tensor("BG", [P, 4], f32, kind="ExternalInput")
    s_d = nc.dram_tensor("S", [P, NCHUNK * P], bf16, kind="ExternalInput")
    idx_d = nc.dram_tensor("IDX", [P, NCHUNK], i32, kind="ExternalInput")
    h2t_d = nc.dram_tensor("H2T", [H, NP], f32, kind="ExternalOutput")

    rg = [list(range(NC))]

    with tile.TileContext(nc) as tc:
        with (
            tc.tile_pool(name="const", bufs=1) as const,
            tc.tile_pool(name="dram", bufs=1, space="DRAM") as dram,
            tc.tile_pool(name="qpool", bufs=4) as qpool,
            tc.tile_pool(name="gpool", bufs=4) as gpool,
            tc.tile_pool(name="psum", bufs=2, space="PSUM") as psum,
        ):
            xT_sb = const.tile([F_IN, NP], bf16)
            wi_sb = const.tile([F_IN, H], bf16)
            bi_sb = const.tile([P, 2], f32)
            wg_sb = const.tile([P, 4 * H], bf16)
            bg_sb = const.tile([P, 4], f32)
            s_sb = const.tile([P, NCHUNK * P], bf16)
            idx_sb = const.tile([P, NCHUNK], i32)
            h0T = const.tile([P, NP], bf16)
            h1T = const.tile([P, NP], bf16)
            hT = (h0T, h1T)


            nc.sync.dma_start(out=wi_sb[:], in_=wi_d[:])
            nc.sync.dma_start(out=bi_sb[:], in_=bi_d[:])
            nc.sync.dma_start(out=wg_sb[:], in_=wg_d[:])
            nc.sync.dma_start(out=bg_sb[:], in_=bg_d[:])
            nc.sync.dma_start(out=xT_sb[:], in_=xT_d[:])
            nc.sync.dma_start(out=idx_sb[:], in_=idx_d[:])
            nc.sync.dma_start(out=s_sb[:], in_=s_d[:])

            # ---- input projection: h0T = relu(Wi.T @ xT + bi) -------------
            for j in range(NPROJ):
                cs = slice(j * WPROJ, (j + 1) * WPROJ)
                for m in range(2):
                    ps_proj = psum.tile([P, WPROJ], f32)
                    nc.tensor.matmul(
                        out=ps_proj[:],
                        lhsT=wi_sb[:, m * P : (m + 1) * P],
                        rhs=xT_sb[:, cs],
                        start=True,
                        stop=True,
                    )
                    nc.scalar.activation(
                        out=hT[m][:, cs],
                        in_=ps_proj[:],
                        func=mybir.ActivationFunctionType.Relu,
                        bias=bi_sb[:, m : m + 1],
                        scale=1.0,
                    )

            # ---- two full GCN conv layers (conv block repeated `reps`x) ---
            for rep, layer in [(r, l) for r in range(reps) for l in range(2)]:
                to_dram = layer == 1 and rep == reps - 1
                cc_in = dram.tile([NP, H], bf16, tag=f"cc{rep}_{layer}")
                qf = dram.tile(
                    [NC * NP, H], bf16, addr_space="Shared", tag=f"qf{rep}_{layer}"
                )

                # dense: q = h @ Wg[layer]  (node-major out, bf16, to DRAM)
                for nt in range(NT):
                    ps_q = psum.tile([P, H], f32)
                    for fc in range(2):
                        nc.tensor.matmul(
                            out=ps_q[:],
                            lhsT=hT[fc][:, nt * P : (nt + 1) * P],
                            rhs=wg_sb[:, (2 * layer + fc) * H : (2 * layer + fc + 1) * H],
                            start=(fc == 0),
                            stop=(fc == 1),
                        )
                    q_sb = qpool.tile([P, H], bf16)
                    nc.vector.tensor_copy(out=q_sb[:], in_=ps_q[:])
                    nc.sync.dma_start(
                        out=cc_in[nt * P : (nt + 1) * P, :], in_=q_sb[:]
                    )

                if variant != "dense":
                    nc.gpsimd.collective_compute(
                        "AllGather",
                        mybir.AluOpType.bypass,
                        replica_groups=rg,
                        ins=[cc_in.opt()],
                        outs=[qf.opt()],
                    )

                if variant in ("noagg", "dense"):
                    for t in range(NT):
                        ts = slice(t * P, (t + 1) * P)
                        for fc in range(2):
                            q2 = qpool.tile([P, P], f32 if to_dram else bf16)
                            nc.vector.tensor_copy(
                                out=q2[:], in_=s_sb[:, t * P : (t + 1) * P]
                            )
                            if not to_dram:
                                nc.vector.tensor_copy(out=hT[fc][:, ts], in_=q2[:])
                            else:
                                nc.sync.dma_start(
                                    out=h2t_d[fc * P : (fc + 1) * P, ts], in_=q2[:]
                                )
                    continue

                gsrc = cc_in if variant == "nocoll" else qf

                # gather + aggregate: h_next.T[:, tile] = sum_k Qg_k.T @ S_k
                # One wide indirect DMA per dest tile: C index columns gather
                # C*128 rows into g[p, k*H:(k+1)*H] = src[idx[p, t*C+k], :].
                for t in range(NT):
                    g = gpool.tile([P, C * H], bf16)
                    if variant == "noindirect":
                        for k in range(C):
                            rr = ((t * C + k) * P) % (
                                NP * (1 if gsrc is cc_in else NC) - P
                            )
                            nc.sync.dma_start(
                                out=g[:, k * H : (k + 1) * H],
                                in_=gsrc[rr : rr + P, :],
                            )
                    else:
                        nc.gpsimd.indirect_dma_start(
                            out=g[:],
                            out_offset=None,
                            in_=gsrc[:],
                            in_offset=bass.IndirectOffsetOnAxis(
                                ap=idx_sb[:, t * C : (t + 1) * C], axis=0
                            ),
                        )
                    ps_a = psum.tile([P, P], f32)
                    ps_b = psum.tile([P, P], f32)
                    for k in range(C):
                        jj = t * C + k
                        nc.tensor.matmul(
                            out=ps_a[:],
                            lhsT=g[:, k * H : k * H + P],
                            rhs=s_sb[:, jj * P : (jj + 1) * P],
                            start=(k == 0),
                            stop=(k == C - 1),
                        )
                        nc.tensor.matmul(
                            out=ps_b[:],
                            lhsT=g[:, k * H + P : (k + 1) * H],
                            rhs=s_sb[:, jj * P : (jj + 1) * P],
                            start=(k == 0),
                            stop=(k == C - 1),
                        )
                    ts = slice(t * P, (t + 1) * P)
                    for fc, pp in ((0, ps_a), (1, ps_b)):
                        bias_ap = bg_sb[:, 2 * layer + fc : 2 * layer + fc + 1]
                        if not to_dram:
                            nc.scalar.activation(
                                out=hT[fc][:, ts],
                                in_=pp[:],
                                func=mybir.ActivationFunctionType.Relu,
                                bias=bias_ap,
                                scale=1.0,
                            )
                        else:
                            h2_sb = qpool.tile([P, P], f32)
                            nc.scalar.activation(
                                out=h2_sb[:],
                                in_=pp[:],
                                func=mybir.ActivationFunctionType.Relu,
                                bias=bias_ap,
                                scale=1.0,
                            )
                            nc.sync.dma_start(
                                out=h2t_d[fc * P : (fc + 1) * P, ts], in_=h2_sb[:]
                            )

    nc.compile()
    return nc


def _preprocess(x, edge_index, edge_attr):
    src = np.asarray(edge_index[0], dtype=np.int64)
    dst = np.asarray(edge_index[1], dtype=np.int64)
    ew = np.asarray(edge_attr, dtype=np.float64)

    loop = np.arange(N, dtype=np.int64)
    src = np.concatenate([src, loop])
    dst = np.concatenate([dst, loop])
    ew = np.concatenate([ew, np.ones(N)])

    deg = np.zeros(N)
    np.add.at(deg, dst, ew)
    dinv = np.where(deg > 0, 1.0 / np.sqrt(np.maximum(deg, 1e-30)), 0.0)
    norm = dinv[src] * ew * dinv[dst]

    # group edges by (dest core, dest tile)
    dslot = dst % NSH
    key = (dst // NSH) * NT + dslot // P
    dcol = dslot % P
    gp_src = (src // NSH) * NP + (src % NSH)  # padded global row in Q_full

    order = np.argsort(key, kind="stable")
    key_s = key[order]
    counts = np.bincount(key_s, minlength=NC * NT)
    starts = np.zeros(NC * NT + 1, dtype=np.int64)
    np.cumsum(counts, out=starts[1:])
    pos = np.arange(len(key_s)) - starts[key_s]

    C = max(1, int(np.ceil(counts.max() / P)))
    cap = C * P
    src_pad = np.zeros((NC * NT, cap), np.int32)
    norm_pad = np.zeros((NC * NT, cap), np.float32)
    dcol_pad = np.zeros((NC * NT, cap), np.int32)
    src_pad[key_s, pos] = gp_src[order]
    norm_pad[key_s, pos] = norm[order]
    dcol_pad[key_s, pos] = dcol[order]

    return src_pad, norm_pad, dcol_pad, C, (src, dst, norm)


def _make_in_maps(x, src_pad, norm_pad, dcol_pad, C, Wi, bi, Wg, bg):
    x = np.asarray(x, dtype=np.float32)
    Wi = np.asarray(Wi, dtype=np.float32)
    bi = np.asarray(bi, dtype=np.float32)
    Wg = np.asarray(Wg, dtype=np.float32)
    bg = np.asarray(bg, dtype=np.float32)

    wi_dev = Wi.astype(BF16)
    bi_dev = np.ascontiguousarray(bi.reshape(2, P).T)
    wg_dev = np.concatenate(
        [Wg[i, fc * P : (fc + 1) * P, :] for i in range(2) for fc in range(2)],
        axis=1,
    ).astype(BF16)
    bg_dev = np.ascontiguousarray(
        np.stack(
            [bg[i, fc * P : (fc + 1) * P] for i in range(2) for fc in range(2)],
            axis=1,
        )
    )

    in_maps = []
    ar_chunk = np.arange(NT * C)[:, None]
    ar_p = np.arange(P)[None, :]
    for c in range(NC):
        rows = slice(c * NT, (c + 1) * NT)
        sp = src_pad[rows].reshape(NT * C, P)
        npd = norm_pad[rows].reshape(NT * C, P)
        dc = dcol_pad[rows].reshape(NT * C, P)
        S = np.zeros((NT * C, P, P), np.float32)
        S[ar_chunk, ar_p, dc] = npd
        s_dev = np.ascontiguousarray(S.transpose(1, 0, 2).reshape(P, NT * C * P)).astype(BF16)
        idx_dev = np.ascontiguousarray(sp.T)

        xT = np.zeros((F_IN, NP), np.float32)
        xT[:, :NSH] = x[c * NSH : (c + 1) * NSH].T
        in_maps.append(
            {
                "xT": xT.astype(BF16),
                "WI": wi_dev,
                "BI": bi_dev,
                "WG": wg_dev,
                "BG": bg_dev,
                "S": s_dev,
                "IDX": idx_dev,
            }
        )
    return in_maps


def _host_head(results, edges, t_idx, Wg, bg, Wo1, bo1, Wo2, bo2,
               Wh1, bh1, Wh2, bh2, Wh3, bh3):
    src, dst, norm = edges
    mask = dst == t_idx
    e_src = src[mask]
    e_norm = norm[mask]

    hsum = np.zeros(H, dtype=np.float64)
    for s, w in zip(e_src, e_norm):
        c = int(s // NSH)
        slot = int(s % NSH)
        hsum += w * results[c]["H2T"][:, slot].astype(np.float64)

    f8 = lambda a: np.asarray(a, dtype=np.float64)
    t = hsum @ f8(Wg)[2] + f8(bg)[2]
    t = np.maximum(t @ f8(Wo1) + f8(bo1), 0.0)
    t = t @ f8(Wo2) + f8(bo2)
    t = np.maximum(t @ f8(Wh1) + f8(bh1), 0.0)
    t = np.maximum(t @ f8(Wh2) + f8(bh2), 0.0)
    return (t @ f8(Wh3) + f8(bh3)).astype(np.float32)


def _run(inputs, trace=False, trace_kwargs=None, variant="full"):
    x = np.asarray(inputs["x"])
    t_idx = int(np.asarray(inputs["target_species_idx"]))

    src_pad, norm_pad, dcol_pad, C, edges = _preprocess(
        x, np.asarray(inputs["edge_index"]), np.asarray(inputs["edge_attr"])
    )

    if (C, variant) not in _PROGRAM_CACHE:
        _PROGRAM_CACHE[(C, variant)] = _build_program(C, variant)
    nc = _PROGRAM_CACHE[(C, variant)]

    in_maps = _make_in_maps(
        x, src_pad, norm_pad, dcol_pad, C,
        inputs["Wi"], inputs["bi"], inputs["Wg"], inputs["bg"],
    )

    res = run_bass_kernel_spmd(
        nc, in_maps, list(range(NC)), trace=trace, **(trace_kwargs or {})
    )

    out = _host_head(
        res.results, edges, t_idx,
        inputs["Wg"], inputs["bg"], inputs["Wo1"], inputs["bo1"],
        inputs["Wo2"], inputs["bo2"], inputs["Wh1"], inputs["bh1"],
        inputs["Wh2"], inputs["bh2"], inputs["Wh3"], inputs["bh3"],
    )
    return out, res


def kernel(**inputs):
    out, _ = _run(inputs)
    return out

